# revision 1
# baseline (speedup 1.0000x reference)
"""NSA-style compressed + top-k block-sparse attention (MiniCPMSparseFlashAttention2)
for Trainium2, distributed over 8 NeuronCores.

Key reduction (validated against the reference): with KERNEL=32, STRIDE=16,
BLOCK=64, TOPK=4, INIT_BLOCKS=1, LOCAL_BLOCKS=2, the reference top-k selection
collapses to: query block qb attends to key blocks
    qb=0 -> {} (output exactly 0), qb=1 -> {0}, qb=2 -> {0,1},
    qb>=3 -> {0, qb-1, b*} where b* = argmax over b in [1, qb-2] of the
    max-pooled compressed-attention (stage 1) score.
So only ONE data-dependent block per query. Stage 2 attends over at most 192
keys; the b* block is sliced straight out of SBUF with register-indexed
(dynamic) access patterns on the matmul moving operand - no gather DMAs.

Per-query matmul outputs land in sg-row order via descending prefix-overwrite:
emit per-query matmuls for query i = 15..0 with lhsT = q columns [0, 8i+8) and
start=True, so query i's rows [8i, 8i+8) are written last by its own matmul
(PSUM base stays 0, which the hardware requires).

Sharding: 8 cores = 2 KV heads x 4 query-block interleaves (core part p owns
query blocks p, p+4, ..., p+28 - balanced). One uniform program for all cores;
all per-core structure flows through input tables, never through code.
"""
import sys
sys.path.insert(0, '/opt/trn_rl_repo')
import math
import re as _re
import numpy as np

import concourse.bass as bass
import concourse.tile as tile
import concourse.mybir as mybir
from concourse import masks
from concourse.bass_utils import run_bass_kernel_spmd

dt = mybir.dt
F32 = dt.float32
I32 = dt.int32
AF = mybir.ActivationFunctionType
ALU = mybir.AluOpType
AX = mybir.AxisListType

S, HQ, HKV, D = 2048, 16, 2, 128
G = HQ // HKV                     # 8 query heads per kv head
KERNEL, STRIDE, BLOCK = 32, 16, 64
B = S // BLOCK                    # 32 kv blocks
C = (S - KERNEL) // STRIDE + 1    # 127 compressed keys
NCORES = 8
NPART = 4                         # seq-parallel parts per kv head
NQ = S // NPART                   # 512 queries per core
NCH = NQ * G // 128               # 32 chunks of 128 (query,g) rows
NST = NQ // 128                   # 4 score tiles of 128 queries
QPC = 16                          # queries per chunk
SCALE = 1.0 / math.sqrt(D)
SCALE1 = SCALE / KERNEL           # stage-1 kcmp left unnormalized (sum not mean)
NEG = -1e30


# ---------------------------------------------------------------------------
# Two workarounds for this container's bass/walrus build.
#
# 1. Tail drain: TileContext's exit drain carries all end-of-kernel sem waits
#    on one CTRL Drain instruction, which this walrus rejects ("Too many sync
#    wait commands"). Emit the waits as separate SP wait_ge instructions and a
#    bare drain instead.
#
# 2. Register recycling: every matmul whose access pattern carries a register
#    offset permanently leaks PE scratch registers ("PE_tmp_<id>") during
#    lowering; the pool holds ~48, capping a kernel at ~16 dynamic matmuls.
#    Freeing the scratch registers after each instruction commits is safe:
#    lowering runs in final scheduled order, the reg-compute writes are
#    emitted immediately before their consumer, and the PE queue executes in
#    order - a later instruction reusing the physical register writes it
#    strictly after the earlier consumer read it. The value cache is
#    barriered so no stale cached lowering references a recycled register.
import bass_rust as _bass_rust


def _patched_drain_and_barrier(self, tick_clock, wait_clock):
    nc = self.nc
    ticks = [int(v) for v in _re.findall(r"-?\d+", repr(tick_clock.global_clock))]
    sems = self.sems.allocated()
    for proc, sem in sems.items():
        t = ticks[proc]
        if t > 0:
            nc.sync.wait_ge(sem, t * (16 if "DMA" in sem.name else 1))
    nc.sync.drain()
    nc.all_engine_barrier()
    popped = nc._tile_sem_poison_stack.pop()
    assert popped is self._sem_poison
    nc.clear_and_free_semaphores(list(sems.values()))
    nc.all_engine_barrier()


_orig_commit = tile.TileContext._commit_instruction


def _has_reg_ap(inst):
    for a in list(getattr(inst, "ins", ())) + list(getattr(inst, "outs", ())):
        if isinstance(a, mybir.RegisterAccessPattern):
            return True
    return False


def _patched_commit_instruction(self, inst, lazy_reg_writes=True):
    r = _orig_commit(self, inst, lazy_reg_writes)
    if _has_reg_ap(inst):
        nc = self.nc
        st = nc._state
        wm = getattr(self, "_regfree_watermark", 0)
        cur = nc.next_id()
        freed_any = False
        for i in range(wm, cur):
            for nm in (f"PE_tmp_{i}", f"PE_PE_roff_snap_{i}"):
                try:
                    if st.lookup_reg(nm) is None:
                        continue
                except Exception:
                    continue
                try:
                    st.free_register(
                        _bass_rust.RegisterHandle(nm, mybir.EngineType.PE))
                    freed_any = True
                except Exception:
                    pass
        self._regfree_watermark = cur
        if freed_any:
            nc.tensor.get_value_cache().barrier()
    return r


tile.TileContext._drain_and_barrier = _patched_drain_and_barrier
tile.TileContext._commit_instruction = _patched_commit_instruction


def _split_excess_waits(nc, keep=1):
    """Walrus here rejects >1 sync wait on several instruction encodings
    (CTRL drain, S3_LW matmul). Move excess waits onto injected same-engine
    InstEventSemaphore instructions placed immediately before."""
    for f in nc.m.functions:
        for bb in f.blocks:
            old = list(bb.instructions)
            if not any(i.sync_info and i.sync_info.on_wait and
                       len(i.sync_info.on_wait) > keep for i in old):
                continue
            new = []
            for inst in old:
                si = inst.sync_info
                if si and si.on_wait and len(si.on_wait) > keep:
                    waits = list(si.on_wait)
                    excess, kept = waits[:-keep], waits[-keep:]
                    for w in excess:
                        new.append(mybir.InstEventSemaphore(
                            name=nc.get_next_instruction_name(),
                            engine=inst.engine, ins=[], outs=[],
                            sync_info=mybir.SyncInfo(on_wait=[w], on_update=[]),
                        ))
                    inst.sync_info = mybir.SyncInfo(
                        on_wait=kept, on_update=list(si.on_update))
                new.append(inst)
            bb.instructions = new


def build_program():
    nc = bass.Bass("TRN2", num_devices=NCORES)
    tensors = dict(
        qT=nc.dram_tensor("qT", [128, NQ * G], F32, kind="ExternalInput"),
        kT=nc.dram_tensor("kT", [128, S], F32, kind="ExternalInput"),
        vb=nc.dram_tensor("vb", [64, B * 128], F32, kind="ExternalInput"),
        kfix2=nc.dram_tensor("kfix2", [128, NCH * 64], F32, kind="ExternalInput"),
        vfix2=nc.dram_tensor("vfix2", [64, NCH * 128], F32, kind="ExternalInput"),
        bias1=nc.dram_tensor("bias1", [128, NCH * C], F32, kind="ExternalInput"),
        m2a0=nc.dram_tensor("m2a0", [128, NCH], F32, kind="ExternalInput"),
        m2a1=nc.dram_tensor("m2a1", [128, NCH], F32, kind="ExternalInput"),
        m2b=nc.dram_tensor("m2b", [128, NCH], F32, kind="ExternalInput"),
        mmid=nc.dram_tensor("mmid", [128, NST * B], F32, kind="ExternalInput"),
        iotab=nc.dram_tensor("iotab", [128, B], F32, kind="ExternalInput"),
        aones48=nc.dram_tensor("aones48", [128, 48], F32, kind="ExternalInput"),
        out=nc.dram_tensor("out", [NQ * G, D], F32, kind="ExternalOutput"),
    )
    with tile.TileContext(nc) as tc:
        _build_body(nc, tc, tensors)
    _split_excess_waits(nc)
    return nc


def _build_body(nc, tc, t):
    from contextlib import ExitStack
    with ExitStack() as ctx:
        const = ctx.enter_context(tc.tile_pool(name="const", bufs=1))

        qT = const.tile([128, NQ * G], F32)
        kT = const.tile([128, S], F32)
        vb = const.tile([64, B * 128], F32)
        kfix2 = const.tile([128, NCH * 64], F32)
        vfix2 = const.tile([64, NCH * 128], F32)
        bias1 = const.tile([128, NCH * C], F32)
        m2a0 = const.tile([128, NCH], F32)
        m2a1 = const.tile([128, NCH], F32)
        m2b = const.tile([128, NCH], F32)
        mmid = const.tile([128, NST * B], F32)
        iotab = const.tile([128, B], F32)
        for j in range(4):
            nc.sync.dma_start(qT[:, j * 1024:(j + 1) * 1024],
                              t["qT"][:, j * 1024:(j + 1) * 1024])
            nc.sync.dma_start(bias1[:, j * 1016:(j + 1) * 1016],
                              t["bias1"][:, j * 1016:(j + 1) * 1016])
        nc.sync.dma_start(kT[:], t["kT"][:])
        nc.sync.dma_start(vb[:], t["vb"][:])
        nc.sync.dma_start(kfix2[:], t["kfix2"][:])
        nc.sync.dma_start(vfix2[:], t["vfix2"][:])
        nc.sync.dma_start(m2a0[:], t["m2a0"][:])
        nc.sync.dma_start(m2a1[:], t["m2a1"][:])
        nc.sync.dma_start(m2b[:], t["m2b"][:])
        nc.sync.dma_start(mmid[:], t["mmid"][:])
        nc.sync.dma_start(iotab[:], t["iotab"][:])
        out_d = t["out"]

        ident = const.tile([128, 128], F32)
        masks.make_identity(nc, ident[:])
        aones48 = const.tile([128, 48], F32)
        nc.sync.dma_start(aones48[:], t["aones48"][:])

        # ---- compressed keys: kcmpT[d, c] = sum_{j<32} kT[d, 16c+j] -------
        half = const.tile([128, 128], F32)
        nc.vector.tensor_copy(half[:], kT[:, 0:S:16])
        for j in range(1, 16):
            nc.vector.tensor_add(half[:], half[:], kT[:, j:S:16])
        kcmpT = const.tile([128, C], F32)
        nc.vector.tensor_add(kcmpT[:], half[:, 0:C], half[:, 1:C + 1])

        b64i = [const.tile([128, 1], I32, tag=f"b64_{st}", name=f"b64_{st}")
                for st in range(NST)]
        b128i = [const.tile([128, 1], I32, tag=f"b128_{st}", name=f"b128_{st}")
                 for st in range(NST)]
        roff = nc.tensor.alloc_register("roff")

        # ================= stage 1: scores + argmax block ==================
        with tc.tile_pool(name="ps_lg1", bufs=3, space="PSUM") as ps_lg1, \
             tc.tile_pool(name="ps_sc", bufs=2, space="PSUM") as ps_sc, \
             tc.tile_pool(name="s1", bufs=3) as s1, \
             tc.tile_pool(name="s1sc", bufs=2) as s1sc, \
             tc.tile_pool(name="s1b", bufs=4) as s1b:

            score_sb = [None] * NST
            sc_state = {}
            SKEW = 2

            def s1_front(ch):
                lg1 = ps_lg1.tile([128, C], F32, tag="lg1")
                nc.tensor.matmul(lg1[:], qT[:, 128 * ch:128 * ch + 128],
                                 kcmpT[:], start=True, stop=True)
                ml = s1.tile([128, C], F32, tag="ml")
                nc.vector.tensor_add(ml[:], lg1[:], bias1[:, C * ch:C * ch + C])
                e1 = s1.tile([128, C], F32, tag="e1")
                l1 = s1b.tile([128, 1], F32, tag="l1")
                nc.scalar.activation(e1[:], ml[:], AF.Exp, scale=SCALE1,
                                     accum_out=l1[:])
                r1 = s1b.tile([128, 1], F32, tag="r1")
                nc.vector.tensor_scalar_add(l1[:], l1[:], 1e-20)
                nc.vector.reciprocal(r1[:], l1[:])
                p1 = s1.tile([128, C], F32, tag="p1")
                nc.vector.tensor_scalar_mul(p1[:], e1[:], r1[:])
                return p1

            def s1_back(ch, p1):
                st, sub = divmod(ch, 8)
                if sub == 0:
                    score_sb[st] = s1sc.tile([128, C], F32, tag="score", name="score")
                if sub % 2 == 0:
                    sc_state["t"] = ps_sc.tile([32, C], F32, tag="sc", name="sc")
                    nc.tensor.matmul(sc_state["t"][:], aones48[:, 16:48], p1[:],
                                     start=True, stop=False)
                else:
                    sc32 = sc_state["t"]
                    nc.tensor.matmul(sc32[:], aones48[:, 0:32], p1[:],
                                     start=False, stop=True)
                    nc.vector.tensor_copy(
                        score_sb[st][32 * (sub // 2):32 * (sub // 2) + 32, :],
                        sc32[:])
                if sub == 7:
                    _argmax(nc, score_sb[st], st)

            def _argmax(nc_, score, st):
                blk = s1.tile([128, B], F32, tag="blk")
                nc_.vector.tensor_copy(blk[:], score[:, 0:125:4])
                nc_.vector.tensor_tensor(blk[:], blk[:], score[:, 1:126:4],
                                         op=ALU.max)
                nc_.vector.tensor_tensor(blk[:], blk[:], score[:, 2:127:4],
                                         op=ALU.max)
                nc_.vector.tensor_tensor(blk[:, 0:31], blk[:, 0:31],
                                         score[:, 3:127:4], op=ALU.max)
                nc_.vector.tensor_tensor(blk[:, 1:32], blk[:, 1:32],
                                         score[:, 3:127:4], op=ALU.max)
                nc_.vector.tensor_add(blk[:], blk[:], mmid[:, B * st:B * st + B])
                mx = s1b.tile([128, 1], F32, tag="mx")
                nc_.vector.tensor_reduce(mx[:], blk[:], axis=AX.X, op=ALU.max)
                enc = s1.tile([128, B], F32, tag="enc")
                nc_.vector.tensor_scalar(enc[:], blk[:], mx[:], 1024.0,
                                         op0=ALU.is_lt, op1=ALU.mult)
                nc_.vector.tensor_tensor(enc[:], enc[:], iotab[:],
                                         op=ALU.add)
                bsf = s1b.tile([128, 1], F32, tag="bsf")
                nc_.vector.tensor_reduce(bsf[:], enc[:], axis=AX.X, op=ALU.min)
                bscaled = s1b.tile([128, 1], F32, tag="bscaled")
                nc_.vector.tensor_scalar_mul(bscaled[:], bsf[:], 64.0)
                nc_.vector.tensor_copy(b64i[st][:], bscaled[:])
                nc_.vector.tensor_scalar_mul(bscaled[:], bsf[:], 128.0)
                nc_.vector.tensor_copy(b128i[st][:], bscaled[:])

            pending = []
            for ch in range(NCH):
                pending.append((ch, s1_front(ch)))
                if len(pending) > SKEW:
                    c0, p0 = pending.pop(0)
                    s1_back(c0, p0)
            for c0, p0 in pending:
                s1_back(c0, p0)

        # ================= stage 2: block-sparse attention =================
        with tc.tile_pool(name="ps_lg", bufs=3, space="PSUM") as ps_lg, \
             tc.tile_pool(name="ps_pt", bufs=2, space="PSUM") as ps_pt, \
             tc.tile_pool(name="ps_o", bufs=2, space="PSUM") as ps_o, \
             tc.tile_pool(name="s2", bufs=3) as s2, \
             tc.tile_pool(name="s2p", bufs=2) as s2p, \
             tc.tile_pool(name="s2o", bufs=2) as s2o, \
             tc.tile_pool(name="s2b", bufs=4) as s2b:

            def s2_qk(ch):
                st = ch // 8
                lg = ps_lg.tile([128, 192], F32, tag="lg")
                nc.tensor.matmul(lg[:, 0:64], qT[:, 128 * ch:128 * ch + 128],
                                 kT[:, 0:64], start=True, stop=True)
                nc.tensor.matmul(lg[:, 64:128], qT[:, 128 * ch:128 * ch + 128],
                                 kfix2[:, 64 * ch:64 * ch + 64],
                                 start=True, stop=True)
                for i in reversed(range(QPC)):
                    p = (QPC * ch + i) % 128
                    nc.tensor.reg_load(roff, b64i[st][p:p + 1, 0:1])
                    rhs = bass.AP(kT[:].tensor, roff, [[S, 128], [1, 64]])
                    nc.tensor.matmul(lg[0:8 * i + 8, 128:192],
                                     qT[:, 128 * ch:128 * ch + 8 * i + 8],
                                     rhs, start=True, stop=True)
                # exp: 3 column regions with per-chunk masks folded into bias
                e = s2.tile([128, 192], F32, tag="e")
                lf0 = s2b.tile([128, 1], F32, tag="lf0")
                lf1 = s2b.tile([128, 1], F32, tag="lf1")
                lm = s2b.tile([128, 1], F32, tag="lm")
                nc.scalar.activation(e[:, 0:64], lg[:, 0:64], AF.Exp,
                                     scale=SCALE, bias=m2a0[:, ch:ch + 1],
                                     accum_out=lf0[:])
                nc.scalar.activation(e[:, 64:128], lg[:, 64:128], AF.Exp,
                                     scale=SCALE, bias=m2a1[:, ch:ch + 1],
                                     accum_out=lf1[:])
                nc.scalar.activation(e[:, 128:192], lg[:, 128:192], AF.Exp,
                                     scale=SCALE, bias=m2b[:, ch:ch + 1],
                                     accum_out=lm[:])
                r2 = s2b.tile([128, 1], F32, tag="r2")
                nc.vector.tensor_add(r2[:], lf0[:], lf1[:])
                nc.vector.tensor_add(r2[:], r2[:], lm[:])
                nc.vector.tensor_scalar_add(r2[:], r2[:], 1e-20)
                nc.vector.reciprocal(r2[:], r2[:])
                return e, r2

            def s2_tr(ch, e):
                # all three 64-key groups transposed to partition base 0 so
                # PV stationaries share the moving operand's base partition
                pt_ps = ps_pt.tile([64, 384], F32, tag="pt")
                nc.tensor.transpose(pt_ps[:, 0:128], e[:, 0:64], ident[:])
                nc.tensor.transpose(pt_ps[:, 128:256], e[:, 64:128], ident[:])
                nc.tensor.transpose(pt_ps[:, 256:384], e[:, 128:192], ident[:])
                pt = s2p.tile([64, 384], F32, tag="pt_sb")
                nc.vector.tensor_copy(pt[:], pt_ps[:])
                return pt

            def s2_pv(ch, pt, r2):
                st = ch // 8
                o_ps = ps_o.tile([128, 128], F32, tag="o")
                for i in reversed(range(QPC)):
                    p = (QPC * ch + i) % 128
                    nc.tensor.reg_load(roff, b128i[st][p:p + 1, 0:1])
                    rhs = bass.AP(vb[:].tensor, roff, [[B * 128, 64], [1, 128]])
                    nc.tensor.matmul(o_ps[0:8 * i + 8, :],
                                     pt[:, 256:256 + 8 * i + 8],
                                     rhs, start=True, stop=False)
                nc.tensor.matmul(o_ps[:], pt[:, 0:128], vb[:, 0:128],
                                 start=False, stop=False)
                nc.tensor.matmul(o_ps[:], pt[:, 128:256],
                                 vfix2[:, 128 * ch:128 * ch + 128],
                                 start=False, stop=True)
                outb = s2o.tile([128, 128], F32, tag="outb")
                nc.vector.tensor_scalar_mul(outb[:], o_ps[:], r2[:])
                nc.sync.dma_start(out_d[128 * ch:128 * ch + 128, :], outb[:])

            # 3-stage software pipeline: QK(ch) ... TR(ch-1) ... PV(ch-2)
            state = {}
            for ch in range(NCH):
                state[ch] = {}
                e, r2 = s2_qk(ch)
                state[ch]["e"], state[ch]["r2"] = e, r2
                if ch - 1 >= 0:
                    state[ch - 1]["pt"] = s2_tr(ch - 1, state[ch - 1]["e"])
                if ch - 2 >= 0:
                    s2_pv(ch - 2, state[ch - 2]["pt"], state[ch - 2]["r2"])
                    del state[ch - 2]
            for c0 in (NCH - 2, NCH - 1):
                state[c0]["pt"] = s2_tr(c0, state[c0]["e"])
                s2_pv(c0, state[c0]["pt"], state[c0]["r2"])


_NC_CACHE = None


def _get_program():
    global _NC_CACHE
    if _NC_CACHE is None:
        _NC_CACHE = build_program()
    return _NC_CACHE


def _make_core_inputs(q, k, v, h, part):
    qbs = [part + NPART * j for j in range(NQ // BLOCK)]
    ls = np.concatenate([np.arange(BLOCK * b, BLOCK * b + BLOCK) for b in qbs])
    qc = q[ls][:, h * G:(h + 1) * G, :].reshape(NQ * G, D)
    qT = np.ascontiguousarray(qc.T)
    kh = k[:, h, :]
    kT = np.ascontiguousarray(kh.T)
    vh = v[:, h, :]
    vb_ = np.ascontiguousarray(
        vh.reshape(B, 64, D).transpose(1, 0, 2).reshape(64, B * D))

    qb_of_li = ls // BLOCK
    qb_ch = qb_of_li[QPC * np.arange(NCH)]          # qb per chunk
    qbf = np.maximum(qb_ch - 1, 0)

    kfix2 = np.ascontiguousarray(
        np.concatenate([kT[:, 64 * b_:64 * b_ + 64] for b_ in qbf], axis=1))
    vfix2 = np.ascontiguousarray(
        np.concatenate([vb_[:, 128 * b_:128 * b_ + 128] for b_ in qbf], axis=1))

    # stage-1 visibility bias: row r of chunk ch is query li=16ch+r//8;
    # compressed key c visible iff 16c+31 <= s
    rows_s = ls[(QPC * np.arange(NCH)[None, :] + np.arange(128)[:, None] // G)]
    thr = np.floor((rows_s.astype(np.float64) - (KERNEL - 1)) / STRIDE)
    vis = np.arange(C)[None, :, None] <= thr.T[:, None, :]  # [NCH, C, 128]
    bias1 = np.where(vis, 0.0, NEG).astype(np.float32)      # [NCH, C, 128]
    bias1 = np.ascontiguousarray(
        bias1.transpose(2, 0, 1).reshape(128, NCH * C))

    m2a0 = np.where(qb_ch >= 1, 0.0, NEG).astype(np.float32)
    m2a0 = np.broadcast_to(m2a0, (128, NCH)).copy()
    m2a1 = np.where(qb_ch >= 2, 0.0, NEG).astype(np.float32)
    m2a1 = np.broadcast_to(m2a1, (128, NCH)).copy()
    m2b = np.where(qb_ch >= 3, 0.0, NEG).astype(np.float32)
    m2b = np.broadcast_to(m2b, (128, NCH)).copy()

    mmid = np.full((128, NST * B), -1e38, np.float32)
    for sti in range(NST):
        qb_rows = qb_of_li[128 * sti + np.arange(128)]
        allowed = (np.arange(B)[None, :] >= 1) & \
                  (np.arange(B)[None, :] <= qb_rows[:, None] - 2)
        allowed[~allowed.any(axis=1), 1] = True
        mmid[:, B * sti:B * sti + B] = np.where(allowed, 0.0, -1e38)

    iotab = np.broadcast_to(np.arange(B, dtype=np.float32), (128, B)).copy()
    aones48 = np.zeros((128, 48), np.float32)
    for j in range(16):
        aones48[8 * j:8 * j + 8, 16 + j] = 1.0

    return {"qT": qT, "kT": kT, "vb": vb_, "kfix2": kfix2, "vfix2": vfix2,
            "bias1": bias1, "m2a0": m2a0, "m2a1": m2a1, "m2b": m2b,
            "mmid": mmid, "iotab": iotab, "aones48": aones48}, ls


def kernel(q, k, v, _profile=False):
    q = np.asarray(q, dtype=np.float32)
    k = np.asarray(k, dtype=np.float32)
    v = np.asarray(v, dtype=np.float32)
    nc = _get_program()

    in_maps = []
    ls_per_core = []
    for c in range(NCORES):
        h, part = divmod(c, NPART)
        im, ls = _make_core_inputs(q, k, v, h, part)
        in_maps.append(im)
        ls_per_core.append(ls)

    kw = dict(trace=True) if _profile else {}
    res = run_bass_kernel_spmd(nc, in_maps, list(range(NCORES)), **kw)

    out = np.zeros((S, HQ, D), dtype=np.float32)
    for c in range(NCORES):
        h, part = divmod(c, NPART)
        oc = res.results[c]["out"].reshape(NQ, G, D)
        out[ls_per_core[c], h * G:(h + 1) * G, :] = oc
    if _profile:
        return out, res
    return out



# revision 3
# speedup vs baseline: 1.7621x; 1.7621x over previous
"""NSA-style compressed + top-k block-sparse attention (MiniCPMSparseFlashAttention2)
for Trainium2, distributed over 8 NeuronCores.

v2 design (vs baseline): the data-dependent third block per query is no longer
sliced with register-indexed matmul APs (1024 reg_loads + dynamic LDWEIGHTS
serialized the PE at ~823us). Instead, stage-1's argmax feeds an on-device
int16 index table and two GPSIMD dma_gather calls per score tile (128 queries):
  kTg  = transpose-mode gather  -> [128 d, 8192]   (K^T, keys contiguous)
  vg   = normal-mode gather     -> [128 key, 64, 128 d] (V rows, key%128 part)
Stage 2 then runs entirely on static access patterns in bf16:
  - per chunk (128 rows = 16 queries x 8 heads), logits are computed
    TRANSPOSED [keys, rows]: fixed blocks (0, qb-1) as [64,128] tiles, the
    dynamic blocks as 8 query-PAIR matmuls (keys of q_{2m} on partitions 0:64,
    q_{2m+1} on 64:128 - a checkerboard whose complement is zeroed so the
    pair PV matmul contracts cleanly over all 128 partitions).
  - exp via 4 ACT instructions (bias = per-chunk causal/validity masks),
    row denominators via ones-matmuls, output accumulated TRANSPOSED
    [d, rows] in PSUM. Normalization (o/denom) happens on host.
Stage 1 stays fp32 end-to-end so the argmax matches the reference exactly.

Sharding: 8 cores = 2 KV heads x 4 query-block interleaves (core part p owns
query blocks p, p+4, ..., p+28). One uniform program for all cores; per-core
structure flows through input tables only.
"""
import sys
sys.path.insert(0, '/opt/trn_rl_repo')
import math
import re as _re
import numpy as np
import ml_dtypes

import concourse.bass as bass
import concourse.tile as tile
import concourse.mybir as mybir
from concourse import masks
from concourse.bass_utils import run_bass_kernel_spmd
from concourse.library_config import mlp as _mlp_lib

dt = mybir.dt
F32 = dt.float32
BF16 = dt.bfloat16
I16 = dt.int16
AF = mybir.ActivationFunctionType
ALU = mybir.AluOpType
AX = mybir.AxisListType
BF16_NP = ml_dtypes.bfloat16

S, HQ, HKV, D = 2048, 16, 2, 128
G = HQ // HKV                     # 8 query heads per kv head
KERNEL, STRIDE, BLOCK = 32, 16, 64
B = S // BLOCK                    # 32 kv blocks
C = (S - KERNEL) // STRIDE + 1    # 127 compressed keys
NCORES = 8
NPART = 4                         # seq-parallel parts per kv head
NQ = S // NPART                   # 512 queries per core
NCH = NQ * G // 128               # 32 chunks of 128 (query,g) rows
NST = NQ // 128                   # 4 score tiles of 128 queries
TCH = NCH // NST                  # 8 chunks per tile
NIDX = 128 * BLOCK                # 8192 gather indices per tile
QPC = 16                          # queries per chunk
SCALE = 1.0 / math.sqrt(D)
SCALE1 = SCALE / KERNEL           # stage-1 kcmp left unnormalized (sum not mean)
NEG = -1e30
SKEW = 2


# ---------------------------------------------------------------------------
# Container-build workaround: TileContext's exit drain carries all
# end-of-kernel sem waits on one CTRL Drain instruction, which this walrus
# rejects ("Too many sync wait commands"). Emit the waits as separate SP
# wait_ge instructions and a bare drain instead; also split excess waits on
# any other instruction encoding.
def _patched_drain_and_barrier(self, tick_clock, wait_clock):
    nc = self.nc
    ticks = [int(v) for v in _re.findall(r"-?\d+", repr(tick_clock.global_clock))]
    sems = self.sems.allocated()
    for proc, sem in sems.items():
        t = ticks[proc]
        if t > 0:
            nc.sync.wait_ge(sem, t * (16 if "DMA" in sem.name else 1))
    nc.sync.drain()
    nc.all_engine_barrier()
    popped = nc._tile_sem_poison_stack.pop()
    assert popped is self._sem_poison
    nc.clear_and_free_semaphores(list(sems.values()))
    nc.all_engine_barrier()


tile.TileContext._drain_and_barrier = _patched_drain_and_barrier


def _split_excess_waits(nc, keep=1):
    for f in nc.m.functions:
        for bb in f.blocks:
            old = list(bb.instructions)
            if not any(i.sync_info and i.sync_info.on_wait and
                       len(i.sync_info.on_wait) > keep for i in old):
                continue
            new = []
            for inst in old:
                si = inst.sync_info
                if si and si.on_wait and len(si.on_wait) > keep:
                    waits = list(si.on_wait)
                    excess, kept = waits[:-keep], waits[-keep:]
                    for w in excess:
                        new.append(mybir.InstEventSemaphore(
                            name=nc.get_next_instruction_name(),
                            engine=inst.engine, ins=[], outs=[],
                            sync_info=mybir.SyncInfo(on_wait=[w], on_update=[]),
                        ))
                    inst.sync_info = mybir.SyncInfo(
                        on_wait=kept, on_update=list(si.on_update))
                new.append(inst)
            bb.instructions = new


def build_program():
    nc = bass.Bass("TRN2", num_devices=NCORES,
                   dynamic_dma_scratch_size=49152, num_swdge_queues=4)
    tensors = dict(
        qT=nc.dram_tensor("qT", [128, NQ * G], F32, kind="ExternalInput"),
        qTb=nc.dram_tensor("qTb", [128, NQ * G], BF16, kind="ExternalInput"),
        kT=nc.dram_tensor("kT", [128, S], F32, kind="ExternalInput"),
        kb0=nc.dram_tensor("kb0", [128, 64], BF16, kind="ExternalInput"),
        vb0=nc.dram_tensor("vb0", [64, 128], BF16, kind="ExternalInput"),
        kfix2=nc.dram_tensor("kfix2", [128, NCH * 64], BF16, kind="ExternalInput"),
        vfix2=nc.dram_tensor("vfix2", [64, NCH * 128], BF16, kind="ExternalInput"),
        kg16=nc.dram_tensor("kg16", [S // 16, 16 * D], BF16,
                            kind="ExternalInput"),
        vg_h=nc.dram_tensor("vg_h", [S, D], BF16, kind="ExternalInput"),
        ltab=nc.dram_tensor("ltab", [128, 128], F32, kind="ExternalInput"),
        oh32=nc.dram_tensor("oh32", [128, 32], F32, kind="ExternalInput"),
        cp4=nc.dram_tensor("cp4", [128, 1], F32, kind="ExternalInput"),
        bias1=nc.dram_tensor("bias1", [128, NCH * C], F32, kind="ExternalInput"),
        m2a0=nc.dram_tensor("m2a0", [128, NCH], F32, kind="ExternalInput"),
        m2a1=nc.dram_tensor("m2a1", [128, NCH], F32, kind="ExternalInput"),
        m2b=nc.dram_tensor("m2b", [128, NCH], F32, kind="ExternalInput"),
        mmid=nc.dram_tensor("mmid", [128, NST * B], F32, kind="ExternalInput"),
        iotab=nc.dram_tensor("iotab", [128, B], F32, kind="ExternalInput"),
        aones48=nc.dram_tensor("aones48", [128, 48], F32, kind="ExternalInput"),
        ones128=nc.dram_tensor("ones128", [1, 128], F32, kind="ExternalInput"),
        cp16=nc.dram_tensor("cp16", [128, 4], F32, kind="ExternalInput"),
        onesb=nc.dram_tensor("onesb", [128, 1], BF16, kind="ExternalInput"),
        outT=nc.dram_tensor("outT", [128, NQ * G], F32, kind="ExternalOutput"),
        den=nc.dram_tensor("den", [1, NQ * G], F32, kind="ExternalOutput"),
    )
    nc.gpsimd.load_library(_mlp_lib)
    with tile.TileContext(nc) as tc:
        _build_body(nc, tc, tensors)
    mybir.codegen_inst_isa_subclasses(nc)   # encode the library-load ISA op
    _split_excess_waits(nc)
    return nc


def _build_body(nc, tc, t):
    from contextlib import ExitStack
    with ExitStack() as ctx:
        const = ctx.enter_context(tc.tile_pool(name="const", bufs=1))

        qT = const.tile([128, NQ * G], F32)
        qTb = const.tile([128, NQ * G], BF16)
        kT = const.tile([128, S], F32)
        kb0 = const.tile([128, 64], BF16)
        vb0 = const.tile([64, 128], BF16)
        kfix2 = const.tile([128, NCH * 64], BF16)
        vfix2 = const.tile([64, NCH * 128], BF16)
        bias1 = const.tile([128, NCH * C], F32)
        m2a0 = const.tile([128, NCH], F32)
        m2a1 = const.tile([128, NCH], F32)
        m2b = const.tile([128, NCH], F32)
        mmid = const.tile([128, NST * B], F32)
        iotab = const.tile([128, B], F32)
        aones48 = const.tile([128, 48], F32)
        ones128 = const.tile([1, 128], F32)
        cp16 = const.tile([128, 4], F32)
        onesb = const.tile([128, 1], BF16)
        ltab = const.tile([128, 128], F32)
        oh32 = const.tile([128, 32], F32)
        cp4 = const.tile([128, 1], F32)
        den_sb = const.tile([1, NQ * G], F32)

        for j in range(4):
            nc.sync.dma_start(qT[:, j * 1024:(j + 1) * 1024],
                              t["qT"][:, j * 1024:(j + 1) * 1024])
            nc.sync.dma_start(qTb[:, j * 1024:(j + 1) * 1024],
                              t["qTb"][:, j * 1024:(j + 1) * 1024])
            nc.sync.dma_start(bias1[:, j * 1016:(j + 1) * 1016],
                              t["bias1"][:, j * 1016:(j + 1) * 1016])
        nc.sync.dma_start(kT[:], t["kT"][:])
        nc.sync.dma_start(kb0[:], t["kb0"][:])
        nc.sync.dma_start(vb0[:], t["vb0"][:])
        nc.sync.dma_start(kfix2[:], t["kfix2"][:])
        nc.sync.dma_start(vfix2[:], t["vfix2"][:])
        nc.sync.dma_start(m2a0[:], t["m2a0"][:])
        nc.sync.dma_start(m2a1[:], t["m2a1"][:])
        nc.sync.dma_start(m2b[:], t["m2b"][:])
        nc.sync.dma_start(mmid[:], t["mmid"][:])
        nc.sync.dma_start(iotab[:], t["iotab"][:])
        nc.sync.dma_start(aones48[:], t["aones48"][:])
        nc.sync.dma_start(ones128[:], t["ones128"][:])
        nc.sync.dma_start(cp16[:], t["cp16"][:])
        nc.sync.dma_start(onesb[:], t["onesb"][:])
        nc.sync.dma_start(ltab[:], t["ltab"][:])
        nc.sync.dma_start(oh32[:], t["oh32"][:])
        nc.sync.dma_start(cp4[:], t["cp4"][:])
        outT_d = t["outT"]

        ident = const.tile([128, 128], F32)
        masks.make_identity(nc, ident[:])

        # ---- compressed keys: kcmpT[d, c] = sum_{j<32} kT[d, 16c+j] -------
        half = const.tile([128, 128], F32)
        nc.vector.tensor_copy(half[:], kT[:, 0:S:16])
        for j in range(1, 16):
            nc.vector.tensor_add(half[:], half[:], kT[:, j:S:16])
        kcmpT = const.tile([128, C], F32)
        nc.vector.tensor_add(kcmpT[:], half[:, 0:C], half[:, 1:C + 1])

        # PSUM budget is 8 banks; pools allocate bufs per TAG, so keep one
        # tag per pool: lg1(2) + sc(1) + bc(1) + pt(2) + o(1) + d(1) = 8
        ps_lg1 = ctx.enter_context(
            tc.tile_pool(name="ps_lg1", bufs=2, space="PSUM"))
        ps_sc = ctx.enter_context(
            tc.tile_pool(name="ps_sc", bufs=1, space="PSUM"))
        ps_bc = ctx.enter_context(
            tc.tile_pool(name="ps_bc", bufs=1, space="PSUM"))
        ps_t = ctx.enter_context(
            tc.tile_pool(name="ps_t", bufs=2, space="PSUM"))
        ps_o = ctx.enter_context(
            tc.tile_pool(name="ps_o", bufs=1, space="PSUM"))
        ps_d = ctx.enter_context(
            tc.tile_pool(name="ps_d", bufs=1, space="PSUM"))
        s1 = ctx.enter_context(tc.tile_pool(name="s1", bufs=3))
        s1sc = ctx.enter_context(tc.tile_pool(name="s1sc", bufs=2))
        s1b = ctx.enter_context(tc.tile_pool(name="s1b", bufs=4))
        sbb = ctx.enter_context(tc.tile_pool(name="sbb", bufs=2))
        sbi = ctx.enter_context(tc.tile_pool(name="sbi", bufs=2))
        gk = ctx.enter_context(tc.tile_pool(name="gk", bufs=2))
        gv = ctx.enter_context(tc.tile_pool(name="gv", bufs=2))
        s2p = ctx.enter_context(tc.tile_pool(name="s2p", bufs=3))
        s2o = ctx.enter_context(tc.tile_pool(name="s2o", bufs=2))
        s2b = ctx.enter_context(tc.tile_pool(name="s2b", bufs=4))

        score_sb = [None] * NST
        sc_state = {}
        kTg = [None] * NST
        vg = [None] * NST
        r512 = nc.gpsimd.to_reg(512)   # shared num_idxs register

        # ================= stage 1: scores + argmax block ==================
        def s1_front(ch):
            lg1 = ps_lg1.tile([128, C], F32, tag="lg1")
            nc.tensor.matmul(lg1[:], qT[:, 128 * ch:128 * ch + 128],
                             kcmpT[:], start=True, stop=True)
            ml = s1.tile([128, C], F32, tag="ml")
            nc.vector.tensor_add(ml[:], lg1[:], bias1[:, C * ch:C * ch + C])
            e1 = s1.tile([128, C], F32, tag="e1")
            l1 = s1b.tile([128, 1], F32, tag="l1")
            nc.scalar.activation(e1[:], ml[:], AF.Exp, scale=SCALE1,
                                 accum_out=l1[:])
            r1 = s1b.tile([128, 1], F32, tag="r1")
            nc.vector.tensor_scalar_add(l1[:], l1[:], 1e-20)
            nc.vector.reciprocal(r1[:], l1[:])
            p1 = s1.tile([128, C], F32, tag="p1")
            nc.vector.tensor_scalar_mul(p1[:], e1[:], r1[:])
            return p1

        def s1_back(ch, p1):
            st, sub = divmod(ch, TCH)
            if sub == 0:
                score_sb[st] = s1sc.tile([128, C], F32, tag="score",
                                         name="score")
            if sub % 2 == 0:
                sc_state["t"] = ps_sc.tile([32, C], F32, tag="sc", name="sc")
                nc.tensor.matmul(sc_state["t"][:], aones48[:, 16:48], p1[:],
                                 start=True, stop=False)
            else:
                sc32 = sc_state["t"]
                nc.tensor.matmul(sc32[:], aones48[:, 0:32], p1[:],
                                 start=False, stop=True)
                nc.vector.tensor_copy(
                    score_sb[st][32 * (sub // 2):32 * (sub // 2) + 32, :],
                    sc32[:])
            if sub == TCH - 1:
                _argmax_and_idx(score_sb[st], st)

        def _argmax_and_idx(score, st):
            blk = s1.tile([128, B], F32, tag="blk")
            nc.vector.tensor_copy(blk[:], score[:, 0:125:4])
            nc.vector.tensor_tensor(blk[:], blk[:], score[:, 1:126:4],
                                    op=ALU.max)
            nc.vector.tensor_tensor(blk[:], blk[:], score[:, 2:127:4],
                                    op=ALU.max)
            nc.vector.tensor_tensor(blk[:, 0:31], blk[:, 0:31],
                                    score[:, 3:127:4], op=ALU.max)
            nc.vector.tensor_tensor(blk[:, 1:32], blk[:, 1:32],
                                    score[:, 3:127:4], op=ALU.max)
            nc.vector.tensor_add(blk[:], blk[:], mmid[:, B * st:B * st + B])
            mx = s1b.tile([128, 1], F32, tag="mx")
            nc.vector.tensor_reduce(mx[:], blk[:], axis=AX.X, op=ALU.max)
            enc = s1.tile([128, B], F32, tag="enc")
            nc.vector.tensor_scalar(enc[:], blk[:], mx[:], 1024.0,
                                    op0=ALU.is_lt, op1=ALU.mult)
            nc.vector.tensor_tensor(enc[:], enc[:], iotab[:], op=ALU.add)
            bsf = s1b.tile([128, 1], F32, tag="bsf")
            nc.vector.tensor_reduce(bsf[:], enc[:], axis=AX.X, op=ALU.min)
            b64f = sbb.tile([128, 1], F32, tag="b64f")
            nc.vector.tensor_scalar_mul(b64f[:], bsf[:], 64.0)
            b4f = sbb.tile([128, 1], F32, tag="b4f")
            nc.vector.tensor_scalar_mul(b4f[:], bsf[:], 4.0)
            # ---- V index table: idx[p, 4q+m] = b64[q] + 16m + p%16 --------
            # (transpose result, query broadcast, and the K table share one
            # PSUM bank, ordered by the SBUF hops in between)
            bc_ps = ps_bc.tile([128, 160], F32, tag="bcp", name="bcp")
            nc.tensor.transpose(bc_ps[0:1, 0:128], b64f[:], ident[:])
            bT = sbb.tile([1, 128], F32, tag="bT")
            nc.vector.tensor_copy(bT[:], bc_ps[0:1, 0:128])
            nc.tensor.matmul(bc_ps[:, 0:128], ones128[:], bT[:],
                             start=True, stop=True)
            idxf = sbi.tile([128, NIDX // 16], F32, tag="idxf")
            for m in range(4):
                nc.vector.tensor_scalar_add(
                    idxf[:, m::4], bc_ps[:, 0:128], cp16[:, m:m + 1])
            idx16 = sbi.tile([128, NIDX // 16], I16, tag="idx16")
            nc.vector.tensor_copy(idx16[:], idxf[:])
            # ---- K index table (16-row-packed gather, 512 idxs):
            # idxk[p, c] = 4*b[4c + (p%16)//4] + (p%16)%4, built as
            # ltab.T @ (oh32 * b4) + cp4
            r2 = sbb.tile([128, 32], F32, tag="r2")
            nc.vector.tensor_scalar_mul(r2[:], oh32[:], b4f[:])
            nc.tensor.matmul(bc_ps[:, 128:160], ltab[:], r2[:],
                             start=True, stop=True)
            idxkf = sbb.tile([128, 32], F32, tag="idxkf")
            nc.vector.tensor_scalar_add(idxkf[:], bc_ps[:, 128:160],
                                        cp4[:])
            idx16k = sbi.tile([128, 32], I16, tag="idx16k")
            nc.vector.tensor_copy(idx16k[:], idxkf[:])
            # ---- gathers for this tile's 128 queries ----------------------
            # (>512 idxs per dma_gather is broken on this platform; K packs
            # 16 key-rows per element so one 512-idx call covers the tile,
            # V needs key-per-partition so it takes 16 x 512-idx calls)
            kTg[st] = gk.tile([128, 16, 512], BF16, tag="kTg", name="kTg")
            nc.gpsimd.dma_gather(kTg[st][:, :, :], t["kg16"][:], idx16k[:],
                                 512, r512, 16 * D, transpose=True,
                                 queue_num=3)
            vg[st] = gv.tile([128, NIDX // 128, D], BF16, tag="vg", name="vg")
            for g in range(16):
                nc.gpsimd.dma_gather(
                    vg[st][:, 4 * g:4 * g + 4, :], t["vg_h"][:],
                    idx16[:, 32 * g:32 * g + 32], 512, r512, D,
                    transpose=False, queue_num=g % 3)

        def s1_tile(st):
            pending = []
            for sub in range(TCH):
                ch = TCH * st + sub
                pending.append((ch, s1_front(ch)))
                if len(pending) > SKEW:
                    c0, p0 = pending.pop(0)
                    s1_back(c0, p0)
            for c0, p0 in pending:
                s1_back(c0, p0)

        # ================= stage 2: block-sparse attention =================
        def s2_chunk(ch):
            st, chl = divmod(ch, TCH)
            kg = kTg[st]
            vgt = vg[st]
            qcol = 128 * ch
            pt_ps = ps_t.tile([128, 384], F32, tag="pt")
            # fixed blocks 0 and qb-1, logits transposed [keys, rows]
            nc.tensor.matmul(pt_ps[0:64, 0:128], kb0[:],
                             qTb[:, qcol:qcol + 128], start=True, stop=True)
            nc.tensor.matmul(pt_ps[0:64, 128:256],
                             kfix2[:, 64 * ch:64 * ch + 64],
                             qTb[:, qcol:qcol + 128], start=True, stop=True)
            # kTg is [128 d, 16 cc, 512 j] with j = 4*q_tile + c4 and
            # key = 64*b[q] + 16*c4 + cc; walrus only takes single-free-dim
            # stationary APs, so linearize this chunk's keys with one DVE
            # copy, then the pair matmuls slice it contiguously
            kc = s2p.tile([128, 1024], BF16, tag="kc")
            nc.vector.tensor_copy(
                kc[:], bass.AP(kg[:, :, :].tensor, 64 * chl,
                               [[16 * 512, 128], [4, 16], [1, 4], [512, 16]]))
            for m in range(8):
                nc.tensor.matmul(
                    pt_ps[:, 256 + 16 * m:256 + 16 * m + 16],
                    kc[:, 128 * m:128 * m + 128],
                    qTb[:, qcol + 16 * m:qcol + 16 * m + 16],
                    start=True, stop=True)
            p_t = s2p.tile([128, 384], BF16, tag="p_t")
            nc.gpsimd.memzero(p_t[:, 256:384])
            nc.scalar.activation(p_t[0:64, 0:128], pt_ps[0:64, 0:128],
                                 AF.Exp, scale=SCALE,
                                 bias=m2a0[0:64, ch:ch + 1])
            nc.scalar.activation(p_t[0:64, 128:256], pt_ps[0:64, 128:256],
                                 AF.Exp, scale=SCALE,
                                 bias=m2a1[0:64, ch:ch + 1])
            pin1 = pt_ps[0:64, 256:384].rearrange("p (m c) -> p m c", m=8)
            pot1 = p_t[0:64, 256:384].rearrange("p (m c) -> p m c", m=8)
            nc.scalar.activation(pot1[:, :, 0:8], pin1[:, :, 0:8],
                                 AF.Exp, scale=SCALE,
                                 bias=m2b[0:64, ch:ch + 1])
            pin2 = pt_ps[64:128, 256:384].rearrange("p (m c) -> p m c", m=8)
            pot2 = p_t[64:128, 256:384].rearrange("p (m c) -> p m c", m=8)
            nc.scalar.activation(pot2[:, :, 8:16], pin2[:, :, 8:16],
                                 AF.Exp, scale=SCALE,
                                 bias=m2b[64:128, ch:ch + 1])
            # row denominators: three ones-matmuls accumulate into one
            # [1,128] region (keeps DVE reads to a single PSUM operand)
            d_ps = ps_d.tile([1, 128], F32, tag="dp", name="dp")
            nc.tensor.matmul(d_ps[:], onesb[0:64, :], p_t[0:64, 0:128],
                             start=True, stop=False)
            nc.tensor.matmul(d_ps[:], onesb[0:64, :], p_t[0:64, 128:256],
                             start=False, stop=False)
            nc.tensor.matmul(d_ps[:], onesb[:], p_t[:, 256:384],
                             start=False, stop=True)
            nc.vector.tensor_copy(den_sb[0:1, qcol:qcol + 128], d_ps[:])
            # PV, output transposed [d, rows]
            # fixed blocks FIRST: start=True clears has_written for the whole
            # bank, so the full-width fixed matmul must open the group; the
            # per-pair matmuls then accumulate onto their column ranges
            o_ps = ps_o.tile([128, 128], F32, tag="o")
            nc.tensor.matmul(o_ps[:], vb0[:], p_t[0:64, 0:128],
                             start=True, stop=False)
            nc.tensor.matmul(o_ps[:], vfix2[:, 128 * ch:128 * ch + 128],
                             p_t[0:64, 128:256], start=False, stop=False)
            for m in range(8):
                nc.tensor.matmul(
                    o_ps[:, 16 * m:16 * m + 16],
                    vgt[:, 8 * chl + m, :],
                    p_t[:, 256 + 16 * m:256 + 16 * m + 16],
                    start=False, stop=(m == 7))
            outb = s2o.tile([128, 128], F32, tag="outb")
            nc.vector.tensor_copy(outb[:], o_ps[:])
            nc.sync.dma_start(outT_d[:, qcol:qcol + 128], outb[:])

        for st in range(NST):
            s1_tile(st)
            if st >= 1:
                for chl in range(TCH):
                    s2_chunk(TCH * (st - 1) + chl)
        for chl in range(TCH):
            s2_chunk(TCH * (NST - 1) + chl)
        nc.sync.dma_start(t["den"][:], den_sb[:])


_NC_CACHE = None


def _get_program():
    global _NC_CACHE
    if _NC_CACHE is None:
        _NC_CACHE = build_program()
    return _NC_CACHE


def _make_core_inputs(q, k, v, h, part):
    qbs = [part + NPART * j for j in range(NQ // BLOCK)]
    ls = np.concatenate([np.arange(BLOCK * b, BLOCK * b + BLOCK) for b in qbs])
    qc = q[ls][:, h * G:(h + 1) * G, :].reshape(NQ * G, D)
    qT = np.ascontiguousarray(qc.T)
    kh = k[:, h, :]
    kT = np.ascontiguousarray(kh.T)
    vh = v[:, h, :]
    vb_ = np.ascontiguousarray(
        vh.reshape(B, 64, D).transpose(1, 0, 2).reshape(64, B * D))

    qb_of_li = ls // BLOCK
    qb_ch = qb_of_li[QPC * np.arange(NCH)]          # qb per chunk
    qbf = np.maximum(qb_ch - 1, 0)

    kfix2 = np.ascontiguousarray(np.concatenate(
        [kT[:, 64 * b_:64 * b_ + 64] for b_ in qbf], axis=1)).astype(BF16_NP)
    vfix2 = np.ascontiguousarray(np.concatenate(
        [vb_[:, 128 * b_:128 * b_ + 128] for b_ in qbf],
        axis=1)).astype(BF16_NP)

    # stage-1 visibility bias: row r of chunk ch is query li=16ch+r//8;
    # compressed key c visible iff 16c+31 <= s
    rows_s = ls[(QPC * np.arange(NCH)[None, :] + np.arange(128)[:, None] // G)]
    thr = np.floor((rows_s.astype(np.float64) - (KERNEL - 1)) / STRIDE)
    vis = np.arange(C)[None, :, None] <= thr.T[:, None, :]  # [NCH, C, 128]
    bias1 = np.where(vis, 0.0, NEG).astype(np.float32)      # [NCH, C, 128]
    bias1 = np.ascontiguousarray(
        bias1.transpose(2, 0, 1).reshape(128, NCH * C))

    m2a0 = np.where(qb_ch >= 1, 0.0, NEG).astype(np.float32)
    m2a0 = np.broadcast_to(m2a0, (128, NCH)).copy()
    m2a1 = np.where(qb_ch >= 2, 0.0, NEG).astype(np.float32)
    m2a1 = np.broadcast_to(m2a1, (128, NCH)).copy()
    m2b = np.where(qb_ch >= 3, 0.0, NEG).astype(np.float32)
    m2b = np.broadcast_to(m2b, (128, NCH)).copy()

    mmid = np.full((128, NST * B), -1e38, np.float32)
    for sti in range(NST):
        qb_rows = qb_of_li[128 * sti + np.arange(128)]
        allowed = (np.arange(B)[None, :] >= 1) & \
                  (np.arange(B)[None, :] <= qb_rows[:, None] - 2)
        allowed[~allowed.any(axis=1), 1] = True
        mmid[:, B * sti:B * sti + B] = np.where(allowed, 0.0, -1e38)

    iotab = np.broadcast_to(np.arange(B, dtype=np.float32), (128, B)).copy()
    aones48 = np.zeros((128, 48), np.float32)
    for j in range(16):
        aones48[8 * j:8 * j + 8, 16 + j] = 1.0

    ones128 = np.ones((1, 128), np.float32)
    cp16 = (16.0 * np.arange(4)[None, :] +
            (np.arange(128) % 16)[:, None]).astype(np.float32)
    onesb = np.ones((128, 1), BF16_NP)
    # K-index-build constants: ltab[q',p]=1 iff q'%4==(p%16)//4,
    # oh32[q',c]=1 iff q'//4==c, cp4[p]=(p%16)%4
    qp = np.arange(128)
    ltab = (qp[:, None] % 4 == ((qp[None, :] % 16) // 4)).astype(np.float32)
    oh32 = (qp[:, None] // 4 == np.arange(32)[None, :]).astype(np.float32)
    cp4 = ((qp % 16) % 4).astype(np.float32)[:, None]

    return {"qT": qT, "qTb": qT.astype(BF16_NP), "kT": kT,
            "kb0": kT[:, 0:64].astype(BF16_NP),
            "vb0": vb_[:, 0:128].astype(BF16_NP),
            "kfix2": kfix2, "vfix2": vfix2,
            "kg16": np.ascontiguousarray(kh).astype(BF16_NP).reshape(
                S // 16, 16 * D),
            "vg_h": np.ascontiguousarray(vh).astype(BF16_NP),
            "bias1": bias1, "m2a0": m2a0, "m2a1": m2a1, "m2b": m2b,
            "mmid": mmid, "iotab": iotab, "aones48": aones48,
            "ones128": ones128, "cp16": cp16, "onesb": onesb,
            "ltab": ltab, "oh32": oh32, "cp4": cp4}, ls


def kernel(q, k, v, _profile=False):
    q = np.asarray(q, dtype=np.float32)
    k = np.asarray(k, dtype=np.float32)
    v = np.asarray(v, dtype=np.float32)
    nc = _get_program()

    in_maps = []
    ls_per_core = []
    for c in range(NCORES):
        h, part = divmod(c, NPART)
        im, ls = _make_core_inputs(q, k, v, h, part)
        in_maps.append(im)
        ls_per_core.append(ls)

    kw = dict(trace=True) if _profile else {}
    res = run_bass_kernel_spmd(nc, in_maps, list(range(NCORES)), **kw)

    out = np.zeros((S, HQ, D), dtype=np.float32)
    for c in range(NCORES):
        h, part = divmod(c, NPART)
        oT = res.results[c]["outT"]                    # [128 d, NQ*G rows]
        den = res.results[c]["den"][0]                 # [NQ*G]
        oc = (oT / np.where(den > 0, den, 1.0)[None, :]).T
        oc = np.where(den[:, None] > 0, oc, 0.0).reshape(NQ, G, D)
        out[ls_per_core[c], h * G:(h + 1) * G, :] = oc
    if _profile:
        return out, res
    return out
